# revision 16
# baseline (speedup 1.0000x reference)
"""Trainium2 Bass kernel for CrossAttention (B=4, T=2048, S=4096, D=256, H=8, Dh=32).

Sharding: 8 cores = 4 batches x 2 T-halves (each core owns TL=1024 query rows of
one batch, all heads). No collectives: host concatenates.

Per-core dataflow (v3):
  Prologue (serial head ~15us): load x/weights, PE-transpose x, q proj (both
  head groups), context chunk 0 (transpose + k-proj g0 + v-proj).
  Context chunks 1-3 production is INTERLEAVED into pass 0's S-tile loop
  (keeps PE duty ~100% so the HAM clock gate stays at 8/8); k-proj for head
  group 1 interleaves into pass 1.

  Attention: 4 passes over (head group g, T-half th). Per S-tile: 4 score
  matmuls (heads 4g..4g+3) on PE row strips 0/32/64/96 run 4-way concurrent
  (K=32 each) into two [128,1024] PSUM tiles (head pairs); ScalarE exps one
  tile (exact exp activation), DVE the other (Schraudolph fast-exp: int16
  bit-written, read back as fp16), checkerboarded per S-tile. attn@v: per
  head pair, 2 col-tiled (M=33) matmuls accumulate [v|ones]^T @ at into a
  [128,512] PSUM acc; the 33rd row is the softmax denominator for free.
  PSUM: 3-buf score ring (6 banks) + 2 accs (2 banks) = 8 banks.

  Normalization: denominator rows -> DRAM scratch -> partition-broadcast ->
  reciprocal_approx_fast; the numerator*reciprocal runs on GpSimd. g0 norm
  hides under pass 2; g1-th0 norm + output projection of T-half 0 hide under
  pass 3. Final out = outTh^T w_out + b.
"""

import sys

if "/opt/trn_rl_repo" not in sys.path:
    sys.path.insert(0, "/opt/trn_rl_repo")

from contextlib import ExitStack

import numpy as np

import concourse.bass as bass
import concourse.tile as tile
from concourse import bacc
from concourse import mybir
from concourse.bass_utils import run_bass_kernel_spmd

B, T, S, D, H, Dh = 4, 2048, 4096, 256, 8, 32
TL = T // 2  # 1024 query rows per core
NXT = TL // 128  # 8 x tiles
NST = S // 128  # 32 S-tiles
SCALE = Dh ** -0.5
FP = mybir.dt.float32
F16 = mybir.dt.float16
I16 = mybir.dt.int16
VW = H * (Dh + 1)  # 264 packed v' columns per S-tile
LOG2E = 1.4426950408889634
SCH_A = float(1024.0 * LOG2E * SCALE)  # Schraudolph scale (fold attn scale)
SCH_B = 15302.0  # fp16 exponent bias + mean-matching correction
EXPF = mybir.ActivationFunctionType.Exp


def build_bass():
    nc = bacc.Bacc()
    ident_d = nc.declare_dram_parameter("ident", [128, 128], FP, isOutput=False)
    x_d = nc.declare_dram_parameter("x", [TL, D], FP, isOutput=False)
    ctx_d = nc.declare_dram_parameter("context", [S, D], FP, isOutput=False)
    wq_d = nc.declare_dram_parameter("w_q", [D, D], FP, isOutput=False)
    wkv_d = nc.declare_dram_parameter("w_kv", [D, 2 * D], FP, isOutput=False)
    wout_d = nc.declare_dram_parameter("w_out", [D, D], FP, isOutput=False)
    bout_d = nc.declare_dram_parameter("b_out", [1, D], FP, isOutput=False)
    out_d = nc.declare_dram_parameter("out", [TL, D], FP, isOutput=True)
    dnscr = nc.dram_tensor("dnscratch", [H, TL], FP)

    with tile.TileContext(nc) as tc, ExitStack() as ctx:
        consts = ctx.enter_context(tc.tile_pool(name="consts", bufs=1))
        persist = ctx.enter_context(tc.tile_pool(name="persist", bufs=1))
        psum = ctx.enter_context(tc.tile_pool(name="psum", bufs=3, space="PSUM"))
        pacc = ctx.enter_context(tc.tile_pool(name="pacc", bufs=1, space="PSUM"))
        atsp = ctx.enter_context(tc.tile_pool(name="atsp", bufs=5))
        atdp = ctx.enter_context(tc.tile_pool(name="atdp", bufs=5))
        fstage = ctx.enter_context(tc.tile_pool(name="fstage", bufs=4))

        identity = consts.tile([128, 128], FP, tag="identity", name="identity")

        wq = [persist.tile([128, D], F16, tag=f"wq{j}", name=f"wq{j}") for j in range(2)]
        wkv = [persist.tile([128, 2 * D], F16, tag=f"wkv{j}", name=f"wkv{j}") for j in range(2)]
        woutg = [persist.tile([128, D], F16, tag=f"woutg{g}", name=f"woutg{g}") for g in range(2)]
        bias_b = persist.tile([128, D], FP, tag="bias_b", name="bias_b")
        qT2 = [persist.tile([128, TL], F16, tag=f"qT2{g}", name=f"qT2{g}") for g in range(2)]
        # kT2 split per context chunk for fine-grained dependencies
        kT2c = [
            [persist.tile([128, 1024], F16, tag=f"kT2c{g}_{ch}", name=f"kT2c{g}_{ch}") for ch in range(4)]
            for g in range(2)
        ]
        vP = [persist.tile([128, 8 * VW], F16, tag=f"vP{i}", name=f"vP{i}") for i in range(4)]
        # per (g, th) tiles so later passes don't false-depend on earlier writes
        dump4 = [
            [persist.tile([128, 512], F16, tag=f"dump4{g}_{th}", name=f"dump4{g}_{th}") for th in range(2)]
            for g in range(2)
        ]
        den4 = [
            [persist.tile([128, 512], FP, tag=f"den4{g}_{th}", name=f"den4{g}_{th}") for th in range(2)]
            for g in range(2)
        ]
        rcpb4 = [
            [persist.tile([128, 512], FP, tag=f"rcpb4{g}_{th}", name=f"rcpb4{g}_{th}") for th in range(2)]
            for g in range(2)
        ]
        outTh4 = [
            [persist.tile([128, 512], F16, tag=f"outTh4{g}_{th}", name=f"outTh4{g}_{th}") for th in range(2)]
            for g in range(2)
        ]
        xT = [persist.tile([128, TL], F16, tag=f"xT{j}", name=f"xT{j}") for j in range(2)]
        cTc = [
            [persist.tile([128, 1024], F16, tag=f"cTc{kj}_{ch}", name=f"cTc{kj}_{ch}") for ch in range(4)]
            for kj in range(2)
        ]
        x_all = persist.tile([128, NXT, D], FP, tag="x_all", name="x_all")
        c_allc = [persist.tile([128, 8, D], FP, tag=f"c_all{cc}", name=f"c_all{cc}") for cc in range(4)]
        wstage = [persist.tile([128, 3 * D], FP, tag=f"wstage{j}", name=f"wstage{j}") for j in range(2)]
        wso = [persist.tile([128, D], FP, tag=f"wso{g}", name=f"wso{g}") for g in range(2)]

        # ---- Phase 0: loads + fp16 weight conversion ----
        nc.sync.dma_start(out=identity, in_=ident_d[:, :])
        nc.sync.dma_start(out=x_all, in_=x_d.rearrange("(t p) d -> p t d", p=128))
        ctx_r = ctx_d.rearrange("(t p) d -> p t d", p=128)
        for cc in range(4):
            nc.sync.dma_start(out=c_allc[cc], in_=ctx_r[:, 8 * cc : 8 * cc + 8, :])
        for j in range(2):
            nc.sync.dma_start(out=wstage[j][:, 0:D], in_=wq_d[128 * j : 128 * j + 128, :])
            nc.sync.dma_start(out=wstage[j][:, D : 3 * D], in_=wkv_d[128 * j : 128 * j + 128, :])
            nc.vector.tensor_copy(wq[j], wstage[j][:, 0:D])
            nc.vector.tensor_copy(wkv[j], wstage[j][:, D : 3 * D])
        for g in range(2):
            nc.sync.dma_start(out=wso[g], in_=wout_d[128 * g : 128 * g + 128, :])
            nc.vector.tensor_copy(woutg[g], wso[g])
        nc.sync.dma_start(out=bias_b, in_=bout_d[0:1, :].partition_broadcast(128))

        # ---- transposes (PE fp32, cast fp16 on PSUM->SBUF evacuation) ----
        tcount = [0]

        def do_transpose4(src_all, st, dstT):
            pt = psum.tile([128, 512], FP, tag="sc", name="pt")
            for a in range(2):
                for j in range(2):
                    nc.tensor.transpose(
                        pt[:, 256 * a + 128 * j : 256 * a + 128 * j + 128],
                        src_all[:, st + a, 128 * j : 128 * j + 128],
                        identity,
                    )
            pt_r = pt.rearrange("p (a b c) -> p a b c", a=2, b=2, c=128)
            dsts = [
                dstT[0][:, 128 * st : 128 * st + 256].rearrange("p (a c) -> p a c", a=2),
                dstT[1][:, 128 * st : 128 * st + 256].rearrange("p (a c) -> p a c", a=2),
            ]
            if tcount[0] % 2 == 0:
                nc.vector.tensor_copy(dsts[0], pt_r[:, :, 0, :])
                nc.scalar.copy(dsts[1], pt_r[:, :, 1, :])
            else:
                nc.scalar.copy(dsts[0], pt_r[:, :, 0, :])
                nc.vector.tensor_copy(dsts[1], pt_r[:, :, 1, :])
            tcount[0] += 1

        def emit_kproj(ch, g):
            pk = psum.tile([128, 1024], FP, tag="sc", name="pk")
            for nt2 in range(2):
                for kj in range(2):
                    nc.tensor.matmul(
                        pk[:, 512 * nt2 : 512 * nt2 + 512],
                        lhsT=wkv[kj][:, 128 * g : 128 * g + 128],
                        rhs=cTc[kj][ch][:, 512 * nt2 : 512 * nt2 + 512],
                        start=(kj == 0),
                        stop=(kj == 1),
                        skip_group_check=True,
                    )
            if (ch + g) % 2 == 0:
                nc.vector.tensor_copy(kT2c[g][ch], pk)
            else:
                nc.scalar.copy(kT2c[g][ch], pk)

        def emit_vproj(ch, lt):
            pv = psum.tile([128, 2 * D], FP, tag="sc", name="pv")
            for a in range(2):
                for kj in range(2):
                    nc.tensor.matmul(
                        pv[:, D * a : D * a + D],
                        lhsT=cTc[kj][ch][:, 128 * (lt + a) : 128 * (lt + a) + 128],
                        rhs=wkv[kj][:, D : 2 * D],
                        start=(kj == 0),
                        stop=(kj == 1),
                        skip_group_check=True,
                    )
            dst = vP[ch][:, VW * lt : VW * lt + 2 * VW].rearrange(
                "p (l h w) -> p l h w", l=2, h=H
            )[:, :, :, 0:Dh]
            srcv = pv.rearrange("p (l h w) -> p l h w", l=2, h=H)
            if lt % 4 == 0:
                nc.vector.tensor_copy(dst, srcv)
            else:
                nc.scalar.copy(dst, srcv)

        # ---- Phase 1 (serial head): x transpose + q proj + context chunk 0 ----
        for t in range(0, NXT, 2):
            do_transpose4(x_all, t, xT)
        for i in range(4):
            ones_cols = vP[i].rearrange("p (s h w) -> p s h w", s=8, h=H)[:, :, :, Dh : Dh + 1]
            nc.vector.memset(ones_cols, 1.0)
        for g in range(2):
            for nt in range(TL // 512):
                pq = psum.tile([128, 512], FP, tag="sc", name="pq")
                for kj in range(2):
                    nc.tensor.matmul(
                        pq,
                        lhsT=wq[kj][:, 128 * g : 128 * g + 128],
                        rhs=xT[kj][:, 512 * nt : 512 * nt + 512],
                        start=(kj == 0),
                        stop=(kj == 1),
                    )
                if nt % 2 == 0:
                    nc.vector.tensor_copy(qT2[g][:, 512 * nt : 512 * nt + 512], pq)
                else:
                    nc.scalar.copy(qT2[g][:, 512 * nt : 512 * nt + 512], pq)
        # context chunk 0: transposes + k proj (g0) + v proj
        for lt in range(0, 8, 2):
            do_transpose4(c_allc[0], lt, [cTc[0][0], cTc[1][0]])
        emit_kproj(0, 0)
        for lt in range(0, 8, 2):
            emit_vproj(0, lt)

        # production work interleaved into pass 0 (chunks 1-3) and pass 1 (k g1)
        items = []
        for ch in (1, 2, 3):
            for lt in (0, 2, 4, 6):
                items.append(("T", ch, lt))
            items.append(("K", ch, 0))
            for lt in (0, 2, 4, 6):
                items.append(("V", ch, lt))
        n_items = len(items)  # 27

        def emit_item(it):
            kind, ch, arg = it
            if kind == "T":
                do_transpose4(c_allc[ch], arg, [cTc[0][ch], cTc[1][ch]])
            elif kind == "K":
                emit_kproj(ch, arg)
            else:
                emit_vproj(ch, arg)

        def emit_outproj(tt):
            # tt indexes 128-row T chunks; th = tt // 4
            th_ = tt // 4
            fin = psum.tile([128, D], FP, tag="sc", name="fin")
            for g_ in range(2):
                nc.tensor.matmul(
                    fin,
                    lhsT=outTh4[g_][th_][:, 128 * (tt % 4) : 128 * (tt % 4) + 128],
                    rhs=woutg[g_],
                    start=(g_ == 0),
                    stop=(g_ == 1),
                )
            outs = fstage.tile([128, D], FP, tag="outs", name="outs")
            nc.vector.tensor_add(outs, fin, bias_b)
            nc.sync.dma_start(out=out_d[128 * tt : 128 * tt + 128, :], in_=outs)

        # ---- Phase 3: attention (4 passes over (head group g, T-half th)) ----
        for p in range(4):
            g, th = p // 2, p % 2
            h0 = 4 * g
            acc = [
                pacc.tile([128, 512], FP, tag=f"acc{half}", name=f"acc{half}")
                for half in range(2)
            ]

            def emit_av(st, at):
                grp, loc = st // 8, st % 8
                for half in range(2):
                    tile_, is_f16 = at[half]
                    for jj in range(2):
                        h = h0 + 2 * half + jj
                        if is_f16:
                            rhs = tile_[:, 512 * jj : 512 * jj + 512]
                        else:
                            rhs = tile_[:, 512 * jj : 512 * jj + 512].bitcast(F16)
                        nc.tensor.matmul(
                            acc[half][64 * jj : 64 * jj + 33, :],
                            lhsT=vP[grp][:, VW * loc + 33 * h : VW * loc + 33 * h + 33],
                            rhs=rhs,
                            start=(st == 0),
                            stop=(st == NST - 1),
                            skip_group_check=True,
                            tile_position=(0, 64 * jj),
                        )

            pend = []
            emitted = [0]
            for st in range(NST):
                sc = []
                for half in range(2):
                    sct = psum.tile([128, TL], FP, tag="sc", name="sct")
                    for jj in range(2):
                        j = 2 * half + jj
                        nc.tensor.matmul(
                            sct[:, 512 * jj : 512 * jj + 512],
                            lhsT=kT2c[g][st // 8][32 * j : 32 * j + 32, 128 * (st % 8) : 128 * (st % 8) + 128],
                            rhs=qT2[g][32 * j : 32 * j + 32, 512 * th : 512 * th + 512],
                            start=True,
                            stop=True,
                            skip_group_check=True,
                            tile_position=(32 * j, 0),
                        )
                    sc.append(sct)
                at = [None, None]
                sth = st % 2  # which head-pair ScalarE takes this S-tile
                ats = atsp.tile([128, TL], F16, tag="ats", name="ats")
                nc.scalar.activation(ats, sc[sth], EXPF, scale=SCALE)
                at[sth] = (ats, True)
                atd = atdp.tile([128, TL], I16, tag="atd", name="atd")
                nc.vector.tensor_scalar(
                    atd, sc[1 - sth], SCH_A, SCH_B,
                    mybir.AluOpType.mult, mybir.AluOpType.add,
                )
                at[1 - sth] = (atd, False)
                pend.append((st, at))
                if len(pend) > 3:
                    emit_av(*pend.pop(0))
                if p == 0 and st < 24:
                    target = min(n_items, ((st + 1) * n_items) // 24)
                    while emitted[0] < target:
                        emit_item(items[emitted[0]])
                        emitted[0] += 1
                if p == 1 and st % 8 == 4:
                    emit_kproj(st // 8, 1)
                if p == 2:
                    if st == 12:
                        nc.vector.reciprocal_approx_fast(rcpb4[0][0], rcpb4[0][0])
                        nc.gpsimd.tensor_mul(outTh4[0][0], dump4[0][0], rcpb4[0][0])
                    elif st == 14:
                        nc.vector.reciprocal_approx_fast(rcpb4[0][1], rcpb4[0][1])
                        nc.gpsimd.tensor_mul(outTh4[0][1], dump4[0][1], rcpb4[0][1])
                if p == 3:
                    if st == 4:
                        nc.vector.reciprocal_approx_fast(rcpb4[1][0], rcpb4[1][0])
                        nc.gpsimd.tensor_mul(outTh4[1][0], dump4[1][0], rcpb4[1][0])
                    elif st >= 8 and st % 4 == 0:
                        emit_outproj(st // 4 - 2)
            for pe_ in pend:
                emit_av(*pe_)

            # dump numerators + denominators for this (g, th) quarter
            for half in range(2):
                for jj in range(2):
                    j = 2 * half + jj
                    if jj == 0:
                        nc.scalar.copy(dump4[g][th][32 * j : 32 * j + 32, :], acc[half][0:32, :])
                        nc.scalar.copy(den4[g][th][32 * j : 32 * j + 1, :], acc[half][32:33, :])
                    else:
                        nc.vector.tensor_copy(dump4[g][th][32 * j : 32 * j + 32, :], acc[half][64:96, :])
                        nc.vector.tensor_copy(den4[g][th][32 * j : 32 * j + 1, :], acc[half][96:97, :])
                    nc.sync.dma_start(
                        out=dnscr[h0 + j : h0 + j + 1, 512 * th : 512 * th + 512],
                        in_=den4[g][th][32 * j : 32 * j + 1, :],
                    )
            if p == 1:
                for th_ in range(2):
                    for i_ in range(4):
                        nc.sync.dma_start(
                            out=rcpb4[0][th_][32 * i_ : 32 * i_ + 32, :],
                            in_=dnscr[i_ : i_ + 1, 512 * th_ : 512 * th_ + 512].partition_broadcast(32),
                        )
            if p == 2:
                for i_ in range(4):
                    nc.sync.dma_start(
                        out=rcpb4[1][0][32 * i_ : 32 * i_ + 32, :],
                        in_=dnscr[4 + i_ : 5 + i_, 0:512].partition_broadcast(32),
                    )

        # ---- Phase 4: g1-th1 normalization + remaining output projection ----
        for i_ in range(4):
            nc.sync.dma_start(
                out=rcpb4[1][1][32 * i_ : 32 * i_ + 32, :],
                in_=dnscr[4 + i_ : 5 + i_, 512:1024].partition_broadcast(32),
            )
        nc.vector.reciprocal_approx_fast(rcpb4[1][1], rcpb4[1][1])
        nc.gpsimd.tensor_mul(outTh4[1][1], dump4[1][1], rcpb4[1][1])
        for tt in range(4, 8):
            emit_outproj(tt)

    nc.compile()
    return nc


_NC = None


def kernel(**inputs):
    global _NC
    x = np.ascontiguousarray(inputs["x"], dtype=np.float32)
    context = np.ascontiguousarray(inputs["context"], dtype=np.float32)
    w_q = np.ascontiguousarray(inputs["w_q"], dtype=np.float32)
    w_kv = np.ascontiguousarray(inputs["w_kv"], dtype=np.float32)
    w_out = np.ascontiguousarray(inputs["w_out"], dtype=np.float32)
    b_out = np.ascontiguousarray(inputs["b_out"], dtype=np.float32).reshape(1, D)

    if _NC is None:
        _NC = build_bass()
    nc = _NC

    in_maps = []
    for c in range(8):
        b, half = c // 2, c % 2
        in_maps.append(
            {
                "ident": np.eye(128, dtype=np.float32),
                "x": np.ascontiguousarray(x[b, TL * half : TL * half + TL, :]),
                "context": np.ascontiguousarray(context[b]),
                "w_q": w_q,
                "w_kv": w_kv,
                "w_out": w_out,
                "b_out": b_out,
            }
        )
    res = run_bass_kernel_spmd(nc, in_maps, core_ids=list(range(8)))
    out = np.empty((B, T, D), dtype=np.float32)
    for c in range(8):
        b, half = c // 2, c % 2
        out[b, TL * half : TL * half + TL, :] = res.results[c]["out"]
    return out


if __name__ == "__main__":
    rng = np.random.default_rng(0)
    ins = {
        "x": rng.standard_normal((B, T, D), dtype=np.float32),
        "context": rng.standard_normal((B, S, D), dtype=np.float32),
        "w_q": rng.standard_normal((D, D), dtype=np.float32) * D**-0.5,
        "w_kv": rng.standard_normal((D, 2 * D), dtype=np.float32) * D**-0.5,
        "w_out": rng.standard_normal((D, D), dtype=np.float32) * D**-0.5,
        "b_out": rng.standard_normal((D,), dtype=np.float32) * 0.01,
    }
    out = kernel(**ins)
    print(out.shape, out.dtype, np.abs(out).mean())
